# revision 26
# baseline (speedup 1.0000x reference)
"""NonLocalBlock (self-attention over 64x64 image, C=256, D=32) on 8 trn2 cores.

Sharding: data-parallel over B=4 batches x 2-way split of the attention
rows => 8 cores, each producing a [2048, 256] slice. Each core receives its
batch image pre-transposed (fp16) rolled so its own 2048 rows come first,
plus its own half in natural layout (fp16) for the residual.

Device dataflow (v2 — PE-lean / dual-engine softmax):
  q/k projections   PE fp16 -> PSUM fp32 -> SBUF via DMA (fp32), used as
                    float32r in the beta matmuls (1 cyc/row, full precision)
  betaT[m, n] = q_m . k_n            PE f32r, out [128m, 1024n] PSUM tiles
  E = exp(betaT)                     split across ACT (true exp) and DVE
                                     (Schraudolph bit-trick exp:
                                     bf16 bits = int16(l*2^7/ln2 + 16250.22))
  o[n, d]  = sum_m E[m, n] v_aug[m, d]   flipped matmul: lhsT = E chunk
             (v_aug[:,32] == 1 => col 32 of o is the softmax denominator)
  otT = PE transpose of o16 pairs    [128, 128] per 2-chunk pair, then
                                     DMA PSUM->SBUF (no engine time)
  F[n, c] = sum_d otT[d, n] WvDup[d, c]
  out[n, c] = F[n, c] * (1/den_n) + x[n, c]
GPSIMD cannot touch PSUM, so Pool only gets SBUF-only work (residual adds
for odd chunks); the PSUM-heavy softmax splits across ACT and DVE.
"""

from contextlib import ExitStack

import ml_dtypes
import numpy as np

import concourse.bass as bass
import concourse.tile as tile
from concourse import bacc, mybir
from concourse.bass_utils import run_bass_kernel_spmd

B, H, W, C = 4, 64, 64, 256
N = H * W            # 4096 pixels per image
D = 32               # reduced channel dim
NH = N // 2          # rows owned by each core
P = 128
MT = N // P          # 32 query (m) tiles
NG = 2               # n-groups per core (1024 output rows each)
GN = NH // NG        # 1024
HGS = 16             # m-tiles per halfgroup
FP32 = mybir.dt.float32
F32R = mybir.dt.float32r
BF16 = mybir.dt.bfloat16
FP16 = mybir.dt.float16
I16 = mybir.dt.int16
NCORES = 8

# Schraudolph constants for bf16-bits exp: bits = int16(x * 2^7/ln2 + B16)
SCH_A = float(2.0**7 / np.log(2.0))
SCH_B = float(127.0 * 128.0 - 5.78)

LAST_RESULTS = None  # BassKernelResults of the most recent run (for test.py)

# exp engine schedule per halfgroup (16 m-tiles): A=ACT true exp,
# D=DVE schraudolph.  A34/D30 overall.
EXP_SCHED = ["ADADADADADADADAA", "ADADADADADADADAD"]


def _body(ctx, tc, out_d, xh_d, xt_d, xt0_d, w3_d, wv_d, id_d):
    nc = tc.nc
    const = ctx.enter_context(tc.tile_pool(name="const", bufs=1))
    big = ctx.enter_context(tc.tile_pool(name="big", bufs=1))
    expp = ctx.enter_context(tc.tile_pool(name="expp", bufs=52))
    osb = ctx.enter_context(tc.tile_pool(name="osb", bufs=4))
    fin = ctx.enter_context(tc.tile_pool(name="fin", bufs=4))
    outp = ctx.enter_context(tc.tile_pool(name="outp", bufs=2))
    ps_pb = ctx.enter_context(tc.tile_pool(name="ps_pb", bufs=3, space="PSUM"))
    ps_m = ctx.enter_context(tc.tile_pool(name="ps_m", bufs=2, space="PSUM"))

    # ---- DRAM loads (host pre-packed) ----
    w_sb = const.tile([P, 2, 3 * D], FP16)  # [p, ch, (Wf|Wg|Wh)]
    nc.scalar.dma_start(w_sb[:], w3_d.rearrange("c p d -> p c d"))

    xt = big.tile([P, 2, N], FP16)  # xT: [c (2 chunks of 128), m]
    # tiny first piece unlocks the k0/q0 projections early; ch0 on SP,
    # ch1 on ACT
    nc.sync.dma_start(xt[:, 0, 0:512], xt0_d[0, :, :])
    nc.scalar.dma_start(xt[:, 1, 0:512], xt0_d[1, :, :])
    pieces = [(512, 2048), (2048, 4096)]
    for a, b in pieces:
        nc.sync.dma_start(xt[:, 0, a:b], xt_d[0, :, a:b])
        nc.scalar.dma_start(xt[:, 1, a:b], xt_d[1, :, a:b])
    wv_dup = const.tile([P, C], BF16)  # WvAug rows at 0:33 and 64:97
    nc.scalar.dma_start(wv_dup[:], wv_d[:, :])
    ident = const.tile([P, P], BF16)
    nc.scalar.dma_start(ident[:], id_d[:, :])

    qt = big.tile([D, N], FP16)
    kt = big.tile([D, NH], FP16)
    v_sb = big.tile([P, MT, D + 1], BF16)
    # per-pair o accumulators, chunks padded to 64 cols so each pair is a
    # contiguous [128, 128] block for the xbar dma transpose (cols 33..63
    # are zero filler).  Per-pair tiles keep the transpose deps precise.
    o16 = [[big.tile([P, 2, 2 * D], BF16, name=f"o16_{g}_{p}") for p in range(4)]
           for g in range(2)]
    o16b = [big.tile([P, 2, 2 * D], BF16, name=f"o16b_{p}") for p in range(4)]
    nc.gpsimd.memset(v_sb[:, :, D : D + 1], 1.0)
    for g in range(2):
        for p in range(4):
            nc.gpsimd.memset(o16[g][p][:], 0.0)

    # PE p-state warmup on a locally memset tile — starts at t~0, no DMA dep
    wsrc = big.tile([P, D], BF16)
    nc.gpsimd.memset(wsrc[:], 0.125)
    warm = ps_m.tile([P, 64], FP32, tag="m", name="warm")
    for i in range(24):
        nc.tensor.matmul(
            warm[0:D, 0:D], wsrc[:, :], wsrc[:, 0:D],
            start=True, stop=True, skip_group_check=True,
        )
    nc.vector.tensor_copy(v_sb[0:D, 0, 0:D], warm[0:D, 0:D])  # keep it live

    # ---- projections ----
    def proj_tile(wofs, dst, t, cast_eng):
        pp = ps_m.tile([D, 512], FP32, tag="m", name=f"pp{wofs}_{t}")
        for ch in range(2):
            nc.tensor.matmul(
                pp[:], w_sb[:, ch, wofs : wofs + D],
                xt[:, ch, t * 512 : (t + 1) * 512],
                start=(ch == 0), stop=(ch == 1),
            )
        cast_eng(dst[:, t * 512 : (t + 1) * 512], pp[:])

    def v_batch(bt, cast_eng):
        pv = ps_m.tile([P, 4, D], FP32, tag="m", name=f"pv{bt}")
        for j in range(4):
            mt = bt * 4 + j
            for ch in range(2):
                nc.tensor.matmul(
                    pv[:, j, :], xt[:, ch, mt * P : (mt + 1) * P],
                    w_sb[:, ch, 2 * D : 3 * D],
                    start=(ch == 0), stop=(ch == 1),
                )
        cast_eng(v_sb[:, bt * 4 : (bt + 1) * 4, 0:D], pv[:])

    A = nc.scalar.copy
    V = nc.vector.tensor_copy
    # minimal prologue: enough q/k for the first betas; the rest of the
    # projections stream inside the hgk0/hgk1 beta loop
    proj_tile(D, kt, 0, A)
    proj_tile(D, kt, 1, V)
    proj_tile(0, qt, 0, A)
    proj_tile(0, qt, 1, V)

    def late_proj(hgk, i):
        j = hgk * HGS + i
        if j < 2:
            proj_tile(D, kt, 2 + j, A if j % 2 == 0 else V)  # k2 k3
        elif j < 8:
            proj_tile(0, qt, j, A if j % 2 == 0 else V)  # q2..q7
        elif j < 16:
            v_batch(j - 8, V if j % 2 == 0 else A)

    x_half = big.tile([P, NH // P, C], FP16)
    xh_src = xh_d.rearrange("(s p) c -> p s c", p=P)
    for piece in range(2):
        nc.sync.dma_start(
            x_half[:, piece * 8 : (piece + 1) * 8, :],
            xh_src[:, piece * 8 : (piece + 1) * 8, :],
        )

    # ---- pipelined attention ----
    ebs = {}  # (g, mt) -> exp tile

    def beta_exp(g, mt, eng):
        pb = ps_pb.tile([P, 1024], FP32, tag="pb", name=f"pb{g}_{mt}")
        for hf in range(2):
            nc.tensor.matmul(
                pb[:, hf * 512 : (hf + 1) * 512],
                qt[:, mt * P : (mt + 1) * P],
                kt[:, g * GN + hf * 512 : g * GN + (hf + 1) * 512],
                start=True, stop=True,
            )
        eb = expp.tile([P, 1024], BF16, tag="eb", name=f"eb{g}_{mt}")
        if eng == "A":
            nc.scalar.activation(eb[:], pb[:], mybir.ActivationFunctionType.Exp)
        else:
            nc.vector.tensor_scalar(
                eb[:].bitcast(I16), pb[:], SCH_A, SCH_B,
                mybir.AluOpType.mult, mybir.AluOpType.add,
            )
        ebs[(g, mt)] = eb

    def o_chunk(g, s, mts, mode, copy_eng=None, opool=None):
        opool = opool if opool is not None else ps_m
        """Accumulating o matmuls for n-chunk s of group g over m-tiles mts.
        mode 'copy' drains PSUM -> o16[g][s//2]; mode 'second' drains to the
        o16b staging pair, then Pool (SBUF-only) adds it into o16."""
        o_ps = opool.tile(
            [P, D + 1], FP32, tag="m" if opool is ps_m else "pb",
            name=f"o{g}_{s}_{mts[0]}",
        )
        for i, mt in enumerate(mts):
            nc.tensor.matmul(
                o_ps[:],
                ebs[(g, mt)][:, s * P : (s + 1) * P],
                v_sb[:, mt, :],
                start=(i == 0), stop=(i == len(mts) - 1),
            )
        p, j = divmod(s, 2)
        if mode == "copy":
            copy_eng(o16[g][p][:, j, 0 : D + 1], o_ps[:])
        else:
            copy_eng(o16b[p][:, j, 0 : D + 1], o_ps[:])
            nc.gpsimd.tensor_tensor(
                o16[g][p][:, j, 0 : D + 1], o16b[p][:, j, 0 : D + 1],
                o16[g][p][:, j, 0 : D + 1], mybir.AluOpType.add,
            )

    otT = {}     # (g, p) -> SBUF [128, 128] transposed pair tile
    rec_g = {}   # (g, p) -> [128, 2, 1] reciprocals of the pair denominators
    out_sb = [None, None]

    def pair_transpose(g, p, via_pe=False, copy_eng=None):
        """Transpose o16 chunk pair (2p, 2p+1) -> SBUF [128, 128].
        Rows 0:33 = chunk 2p (d, den), rows 64:97 = chunk 2p+1.  via_pe
        avoids the ~2.3us DMA latency on the critical tail path."""
        ot = osb.tile([P, P], BF16, tag="ot", name=f"ot{g}_{p}")
        if via_pe:
            tpp = ps_m.tile([P, P], BF16, tag="m", name=f"tpp{g}_{p}")
            nc.tensor.transpose(tpp[:], o16[g][p][:], ident[:])
            copy_eng(ot[:], tpp[:])
        else:
            nc.scalar.dma_start_transpose(ot[:], o16[g][p][:])
        otT[(g, p)] = ot
        rg = fin.tile([P, 2, 1], FP32, tag="rec", name=f"rec{g}_{p}")
        nc.vector.reciprocal(rg[:], o16[g][p][:, :, D : D + 1])
        rec_g[(g, p)] = rg

    def final_sub(g, s, pool=None):
        p, half = divmod(s, 2)
        pool = pool if pool is not None else ps_m
        f_ps = pool.tile([P, C], FP32, tag=pool is ps_m and "m" or "pb", name=f"f{g}_{s}")
        nc.tensor.matmul(
            f_ps[:],
            otT[(g, p)][64 * half : 64 * half + D + 1, :],
            wv_dup[64 * half : 64 * half + D + 1, :],
            start=True, stop=True,
        )
        if s % 2 == 0:
            nc.vector.scalar_tensor_tensor(
                out_sb[g][:, s, :], f_ps[:], rec_g[(g, p)][:, half, :],
                x_half[:, g * (GN // P) + s, :],
                mybir.AluOpType.mult, mybir.AluOpType.add,
            )
        else:
            # ACT does the division (scale by 1/den); Pool adds the residual
            tmp = fin.tile([P, C], FP32, tag="tmp", name=f"tmp{g}_{s}")
            nc.scalar.activation(
                tmp[:], f_ps[:], mybir.ActivationFunctionType.Copy,
                scale=rec_g[(g, p)][:, half, :],
            )
            nc.gpsimd.tensor_tensor(
                out_sb[g][:, s, :], tmp[:], x_half[:, g * (GN // P) + s, :],
                mybir.AluOpType.add,
            )

    def half_out_dma(g, hh):
        lo = g * (GN // P) + hh * 4
        nc.sync.dma_start(
            out_d.rearrange("(s p) c -> p s c", p=P)[:, lo : lo + 4, :],
            out_sb[g][:, hh * 4 : (hh + 1) * 4, :],
        )

    def quarter_out_dma(g, qq):
        lo = g * (GN // P) + qq * 2
        nc.sync.dma_start(
            out_d.rearrange("(s p) c -> p s c", p=P)[:, lo : lo + 2, :],
            out_sb[g][:, qq * 2 : (qq + 1) * 2, :],
        )

    # Pipeline:
    #   hgk0: betas(g0 h0)
    #   hgk1: betas(g0 h1)
    #   hgk2: betas(g1 h0) + full-group o-chunks of g0 + transposes of g0
    #   hgk3: betas(g1 h1) + h0 o-chunks of g1 + finals of g0
    #   tail: h1 o-chunks of g1 (+add) + transposes/finals of g1
    out_sb[0] = outp.tile([P, NG * 4, C], FP16, tag="osb", name="outsb0")
    out_sb[1] = outp.tile([P, NG * 4, C], FP16, tag="osb", name="outsb1")
    AV = (V, A)
    for hgk in range(4):
        g, h = divmod(hgk, 2)
        for i in range(HGS):
            beta_exp(g, h * HGS + i, EXP_SCHED[h][i])
            if hgk < 2:
                late_proj(hgk, i)
            if hgk == 2:
                if i % 2 == 1:
                    o_chunk(0, i // 2, range(MT), "copy", copy_eng=AV[(i // 2) % 2])
                elif i >= 6 and i % 4 == 2:
                    pair_transpose(0, (i - 6) // 4)
            if hgk == 3:
                if i % 2 == 1:
                    o_chunk(
                        1, i // 2, range(HGS), "copy",
                        copy_eng=AV[(i // 2) % 2], opool=ps_pb,
                    )
                elif i == 0:
                    pair_transpose(0, 3)
                elif 2 <= i <= 8:
                    final_sub(0, i - 2)
                    final_sub(0, i - 1)
                elif i == 10:
                    half_out_dma(0, 0)
                    half_out_dma(0, 1)
    # tail: o-chunk pairs -> pool-add -> PE transpose, then finals with
    # quarter-granular output DMAs
    FP = [ps_pb, ps_m, ps_pb, ps_m, ps_pb, ps_m, ps_pb, ps_m]
    for p in range(4):
        o_chunk(
            1, 2 * p, range(HGS, MT), "second", copy_eng=AV[0],
            opool=ps_pb if p % 2 == 0 else ps_m,
        )
        o_chunk(
            1, 2 * p + 1, range(HGS, MT), "second", copy_eng=AV[1],
            opool=ps_pb if p % 2 == 0 else ps_m,
        )
        pair_transpose(1, p, via_pe=True, copy_eng=AV[p % 2])
    for s in range(8):
        final_sub(1, s, pool=FP[s])
        if s % 2 == 1:
            quarter_out_dma(1, s // 2)


def build_program():
    nc = bacc.Bacc(
        "TRN2",
        target_bir_lowering=False,
        debug=False,
        enable_asserts=False,
        num_devices=NCORES,
    )
    xh_d = nc.dram_tensor("xh", [NH, C], FP16, kind="ExternalInput").ap()
    xt_d = nc.dram_tensor("xt", [2, P, N], FP16, kind="ExternalInput").ap()
    xt0_d = nc.dram_tensor("xt0", [2, P, 512], FP16, kind="ExternalInput").ap()
    id_d = nc.dram_tensor("Ident", [P, P], BF16, kind="ExternalInput").ap()
    w3_d = nc.dram_tensor("W3", [2, P, 3 * D], FP16, kind="ExternalInput").ap()
    wv_d = nc.dram_tensor("WvDup", [P, C], BF16, kind="ExternalInput").ap()
    out_d = nc.dram_tensor("out", [NH, C], FP16, kind="ExternalOutput").ap()

    with tile.TileContext(nc) as tc:
        with ExitStack() as ctx:
            _body(ctx, tc, out_d, xh_d, xt_d, xt0_d, w3_d, wv_d, id_d)
    nc.compile()
    return nc


_CACHE = {}


def _get_program():
    if "nc" not in _CACHE:
        _CACHE["nc"] = build_program()
    return _CACHE["nc"]


def make_in_maps(inputs):
    x = np.ascontiguousarray(np.asarray(inputs["x"], np.float32)).reshape(B, N, C)
    gam = np.float32(np.asarray(inputs["gamma"], np.float32).reshape(()))
    w3 = np.empty((C, 3 * D), np.float16)  # [c, (Wf|Wg|Wh)]
    for j, nm in enumerate(("Wf", "Wg", "Wh")):
        w3[:, j * D : (j + 1) * D] = np.asarray(inputs[nm], np.float32).astype(
            np.float16
        )
    w3 = np.ascontiguousarray(w3.reshape(2, P, 3 * D))
    wv_dup = np.zeros((P, C), ml_dtypes.bfloat16)
    wv16 = (gam * np.asarray(inputs["Wv"], np.float32)).astype(ml_dtypes.bfloat16)
    wv_dup[0:D, :] = wv16
    wv_dup[64 : 64 + D, :] = wv16
    ident = np.eye(P, dtype=ml_dtypes.bfloat16)

    in_maps = []
    for c in range(NCORES):
        b, h = divmod(c, 2)
        if h == 0:
            xb = x[b]
        else:
            xb = np.concatenate([x[b, NH:], x[b, :NH]], axis=0)
        xt = np.ascontiguousarray(xb.T.astype(np.float16).reshape(2, P, N))
        in_maps.append(
            {
                "xh": np.ascontiguousarray(xb[:NH].astype(np.float16)),
                "xt": xt,
                "xt0": np.ascontiguousarray(xt[:, :, 0:512]),
                "Ident": ident,
                "W3": w3,
                "WvDup": wv_dup,
            }
        )
    return in_maps


def kernel(**inputs):
    global LAST_RESULTS
    nc = _get_program()
    in_maps = make_in_maps(inputs)
    res = run_bass_kernel_spmd(nc, in_maps, core_ids=list(range(NCORES)))
    LAST_RESULTS = res
    out = np.empty((B, N, C), np.float32)
    for c in range(NCORES):
        b, h = divmod(c, 2)
        out[b, h * NH : (h + 1) * NH] = res.results[c]["out"].astype(np.float32)
    return out.reshape(B, H, W, C)


# revision 28
# speedup vs baseline: 1.0036x; 1.0036x over previous
"""NonLocalBlock (self-attention over 64x64 image, C=256, D=32) on 8 trn2 cores.

Sharding: data-parallel over B=4 batches x 2-way split of the attention
rows => 8 cores, each producing a [2048, 256] slice. Each core receives its
batch image pre-transposed (fp16) rolled so its own 2048 rows come first,
plus its own half in natural layout (fp16) for the residual.

Device dataflow (v2 — PE-lean / dual-engine softmax):
  q/k projections   PE fp16 -> PSUM fp32 -> SBUF via DMA (fp32), used as
                    float32r in the beta matmuls (1 cyc/row, full precision)
  betaT[m, n] = q_m . k_n            PE f32r, out [128m, 1024n] PSUM tiles
  E = exp(betaT)                     split across ACT (true exp) and DVE
                                     (Schraudolph bit-trick exp:
                                     bf16 bits = int16(l*2^7/ln2 + 16250.22))
  o[n, d]  = sum_m E[m, n] v_aug[m, d]   flipped matmul: lhsT = E chunk
             (v_aug[:,32] == 1 => col 32 of o is the softmax denominator)
  otT = PE transpose of o16 pairs    [128, 128] per 2-chunk pair, then
                                     DMA PSUM->SBUF (no engine time)
  F[n, c] = sum_d otT[d, n] WvDup[d, c]
  out[n, c] = F[n, c] * (1/den_n) + x[n, c]
GPSIMD cannot touch PSUM, so Pool only gets SBUF-only work (residual adds
for odd chunks); the PSUM-heavy softmax splits across ACT and DVE.
"""

from contextlib import ExitStack

import ml_dtypes
import numpy as np

import concourse.bass as bass
import concourse.tile as tile
from concourse import bacc, mybir
from concourse.bass_utils import run_bass_kernel_spmd

B, H, W, C = 4, 64, 64, 256
N = H * W            # 4096 pixels per image
D = 32               # reduced channel dim
NH = N // 2          # rows owned by each core
P = 128
MT = N // P          # 32 query (m) tiles
NG = 2               # n-groups per core (1024 output rows each)
GN = NH // NG        # 1024
HGS = 16             # m-tiles per halfgroup
FP32 = mybir.dt.float32
F32R = mybir.dt.float32r
BF16 = mybir.dt.bfloat16
FP16 = mybir.dt.float16
I16 = mybir.dt.int16
NCORES = 8

# Schraudolph constants for bf16-bits exp: bits = int16(x * 2^7/ln2 + B16)
SCH_A = float(2.0**7 / np.log(2.0))
SCH_B = float(127.0 * 128.0 - 5.78)

LAST_RESULTS = None  # BassKernelResults of the most recent run (for test.py)

# exp engine schedule per halfgroup-index (16 m-tiles each): A=ACT true
# exp, D=DVE schraudolph.  A35/D29 overall.
EXP_SCHED = [
    "ADADADADADADADAA",
    "ADADADADADADADAD",
    "ADADADADADADADAA",
    "ADADADADADADADAA",
]


def _body(ctx, tc, out_d, xh_d, xt_d, xt0_d, w3_d, wv_d, id_d):
    nc = tc.nc
    const = ctx.enter_context(tc.tile_pool(name="const", bufs=1))
    big = ctx.enter_context(tc.tile_pool(name="big", bufs=1))
    expp = ctx.enter_context(tc.tile_pool(name="expp", bufs=52))
    osb = ctx.enter_context(tc.tile_pool(name="osb", bufs=4))
    fin = ctx.enter_context(tc.tile_pool(name="fin", bufs=4))
    outp = ctx.enter_context(tc.tile_pool(name="outp", bufs=2))
    ps_pb = ctx.enter_context(tc.tile_pool(name="ps_pb", bufs=3, space="PSUM"))
    ps_m = ctx.enter_context(tc.tile_pool(name="ps_m", bufs=2, space="PSUM"))

    # ---- DRAM loads (host pre-packed) ----
    w_sb = const.tile([P, 2, 3 * D], FP16)  # [p, ch, (Wf|Wg|Wh)]
    nc.scalar.dma_start(w_sb[:], w3_d.rearrange("c p d -> p c d"))

    xt = big.tile([P, 2, N], FP16)  # xT: [c (2 chunks of 128), m]
    # tiny first piece unlocks the k0/q0 projections early; ch0 on SP,
    # ch1 on ACT
    nc.sync.dma_start(xt[:, 0, 0:512], xt0_d[0, :, :])
    nc.scalar.dma_start(xt[:, 1, 0:512], xt0_d[1, :, :])
    pieces = [(512, 2048), (2048, 4096)]
    for a, b in pieces:
        nc.sync.dma_start(xt[:, 0, a:b], xt_d[0, :, a:b])
        nc.scalar.dma_start(xt[:, 1, a:b], xt_d[1, :, a:b])
    wv_dup = const.tile([P, C], BF16)  # WvAug rows at 0:33 and 64:97
    nc.scalar.dma_start(wv_dup[:], wv_d[:, :])
    ident = const.tile([P, P], BF16)
    nc.scalar.dma_start(ident[:], id_d[:, :])

    qt = big.tile([D, N], FP16)
    kt = big.tile([D, NH], FP16)
    v_sb = big.tile([P, MT, D + 1], BF16)
    # per-pair o accumulators, chunks padded to 64 cols so each pair is a
    # contiguous [128, 128] block for the xbar dma transpose (cols 33..63
    # are zero filler).  Per-pair tiles keep the transpose deps precise.
    o16 = [[big.tile([P, 2, 2 * D], BF16, name=f"o16_{g}_{p}") for p in range(4)]
           for g in range(2)]
    o16b = [big.tile([P, 2, 2 * D], BF16, name=f"o16b_{p}") for p in range(4)]
    nc.gpsimd.memset(v_sb[:, :, D : D + 1], 1.0)
    for g in range(2):
        for p in range(4):
            nc.gpsimd.memset(o16[g][p][:], 0.0)

    # PE p-state warmup on a locally memset tile — starts at t~0, no DMA dep
    wsrc = big.tile([P, D], BF16)
    nc.gpsimd.memset(wsrc[:], 0.125)
    warm = ps_m.tile([P, 64], FP32, tag="m", name="warm")
    for i in range(24):
        nc.tensor.matmul(
            warm[0:D, 0:D], wsrc[:, :], wsrc[:, 0:D],
            start=True, stop=True, skip_group_check=True,
        )
    nc.vector.tensor_copy(v_sb[0:D, 0, 0:D], warm[0:D, 0:D])  # keep it live

    # ---- projections ----
    def proj_tile(wofs, dst, t, cast_eng):
        pp = ps_m.tile([D, 512], FP32, tag="m", name=f"pp{wofs}_{t}")
        for ch in range(2):
            nc.tensor.matmul(
                pp[:], w_sb[:, ch, wofs : wofs + D],
                xt[:, ch, t * 512 : (t + 1) * 512],
                start=(ch == 0), stop=(ch == 1),
            )
        cast_eng(dst[:, t * 512 : (t + 1) * 512], pp[:])

    def v_batch(bt, cast_eng):
        pv = ps_m.tile([P, 4, D], FP32, tag="m", name=f"pv{bt}")
        for j in range(4):
            mt = bt * 4 + j
            for ch in range(2):
                nc.tensor.matmul(
                    pv[:, j, :], xt[:, ch, mt * P : (mt + 1) * P],
                    w_sb[:, ch, 2 * D : 3 * D],
                    start=(ch == 0), stop=(ch == 1),
                )
        cast_eng(v_sb[:, bt * 4 : (bt + 1) * 4, 0:D], pv[:])

    A = nc.scalar.copy
    V = nc.vector.tensor_copy
    # minimal prologue: enough q/k for the first betas; the rest of the
    # projections stream inside the hgk0/hgk1 beta loop
    proj_tile(D, kt, 0, A)
    proj_tile(D, kt, 1, V)
    proj_tile(0, qt, 0, A)
    proj_tile(0, qt, 1, V)

    def late_proj(hgk, i):
        j = hgk * HGS + i
        if j < 2:
            proj_tile(D, kt, 2 + j, A if j % 2 == 0 else V)  # k2 k3
        elif j < 8:
            proj_tile(0, qt, j, A if j % 2 == 0 else V)  # q2..q7
        elif j < 16:
            v_batch(j - 8, V if j % 2 == 0 else A)

    x_half = big.tile([P, NH // P, C], FP16)
    xh_src = xh_d.rearrange("(s p) c -> p s c", p=P)
    for piece in range(2):
        nc.sync.dma_start(
            x_half[:, piece * 8 : (piece + 1) * 8, :],
            xh_src[:, piece * 8 : (piece + 1) * 8, :],
        )

    # ---- pipelined attention ----
    ebs = {}  # (g, mt) -> exp tile

    def beta_exp(g, mt, eng):
        pb = ps_pb.tile([P, 1024], FP32, tag="pb", name=f"pb{g}_{mt}")
        for hf in range(2):
            nc.tensor.matmul(
                pb[:, hf * 512 : (hf + 1) * 512],
                qt[:, mt * P : (mt + 1) * P],
                kt[:, g * GN + hf * 512 : g * GN + (hf + 1) * 512],
                start=True, stop=True,
            )
        eb = expp.tile([P, 1024], BF16, tag="eb", name=f"eb{g}_{mt}")
        if eng == "A":
            nc.scalar.activation(eb[:], pb[:], mybir.ActivationFunctionType.Exp)
        else:
            nc.vector.tensor_scalar(
                eb[:].bitcast(I16), pb[:], SCH_A, SCH_B,
                mybir.AluOpType.mult, mybir.AluOpType.add,
            )
        ebs[(g, mt)] = eb

    def o_chunk(g, s, mts, mode, copy_eng=None, opool=None):
        opool = opool if opool is not None else ps_m
        """Accumulating o matmuls for n-chunk s of group g over m-tiles mts.
        mode 'copy' drains PSUM -> o16[g][s//2]; mode 'second' drains to the
        o16b staging pair, then Pool (SBUF-only) adds it into o16."""
        o_ps = opool.tile(
            [P, D + 1], FP32, tag="m" if opool is ps_m else "pb",
            name=f"o{g}_{s}_{mts[0]}",
        )
        for i, mt in enumerate(mts):
            nc.tensor.matmul(
                o_ps[:],
                ebs[(g, mt)][:, s * P : (s + 1) * P],
                v_sb[:, mt, :],
                start=(i == 0), stop=(i == len(mts) - 1),
            )
        p, j = divmod(s, 2)
        if mode == "copy":
            copy_eng(o16[g][p][:, j, 0 : D + 1], o_ps[:])
        else:
            copy_eng(o16b[p][:, j, 0 : D + 1], o_ps[:])
            nc.gpsimd.tensor_tensor(
                o16[g][p][:, j, 0 : D + 1], o16b[p][:, j, 0 : D + 1],
                o16[g][p][:, j, 0 : D + 1], mybir.AluOpType.add,
            )

    otT = {}     # (g, p) -> SBUF [128, 128] transposed pair tile
    rec_g = {}   # (g, p) -> [128, 2, 1] reciprocals of the pair denominators
    out_sb = [None, None]

    def pair_transpose(g, p, via_pe=False, copy_eng=None):
        """Transpose o16 chunk pair (2p, 2p+1) -> SBUF [128, 128].
        Rows 0:33 = chunk 2p (d, den), rows 64:97 = chunk 2p+1.  via_pe
        avoids the ~2.3us DMA latency on the critical tail path."""
        ot = osb.tile([P, P], BF16, tag="ot", name=f"ot{g}_{p}")
        if via_pe:
            tpp = ps_m.tile([P, P], BF16, tag="m", name=f"tpp{g}_{p}")
            nc.tensor.transpose(tpp[:], o16[g][p][:], ident[:])
            copy_eng(ot[:], tpp[:])
        else:
            nc.scalar.dma_start_transpose(ot[:], o16[g][p][:])
        otT[(g, p)] = ot
        rg = fin.tile([P, 2, 1], FP32, tag="rec", name=f"rec{g}_{p}")
        nc.vector.reciprocal(rg[:], o16[g][p][:, :, D : D + 1])
        rec_g[(g, p)] = rg

    def final_sub(g, s, pool=None):
        p, half = divmod(s, 2)
        pool = pool if pool is not None else ps_m
        f_ps = pool.tile([P, C], FP32, tag=pool is ps_m and "m" or "pb", name=f"f{g}_{s}")
        nc.tensor.matmul(
            f_ps[:],
            otT[(g, p)][64 * half : 64 * half + D + 1, :],
            wv_dup[64 * half : 64 * half + D + 1, :],
            start=True, stop=True,
        )
        if s % 2 == 0 or g == 1:
            nc.vector.scalar_tensor_tensor(
                out_sb[g][:, s, :], f_ps[:], rec_g[(g, p)][:, half, :],
                x_half[:, g * (GN // P) + s, :],
                mybir.AluOpType.mult, mybir.AluOpType.add,
            )
        else:
            # ACT does the division (scale by 1/den); Pool adds the residual
            tmp = fin.tile([P, C], FP32, tag="tmp", name=f"tmp{g}_{s}")
            nc.scalar.activation(
                tmp[:], f_ps[:], mybir.ActivationFunctionType.Copy,
                scale=rec_g[(g, p)][:, half, :],
            )
            nc.gpsimd.tensor_tensor(
                out_sb[g][:, s, :], tmp[:], x_half[:, g * (GN // P) + s, :],
                mybir.AluOpType.add,
            )

    def half_out_dma(g, hh):
        lo = g * (GN // P) + hh * 4
        nc.sync.dma_start(
            out_d.rearrange("(s p) c -> p s c", p=P)[:, lo : lo + 4, :],
            out_sb[g][:, hh * 4 : (hh + 1) * 4, :],
        )

    def quarter_out_dma(g, qq):
        lo = g * (GN // P) + qq * 2
        nc.sync.dma_start(
            out_d.rearrange("(s p) c -> p s c", p=P)[:, lo : lo + 2, :],
            out_sb[g][:, qq * 2 : (qq + 1) * 2, :],
        )

    # Pipeline:
    #   hgk0: betas(g0 h0)
    #   hgk1: betas(g0 h1)
    #   hgk2: betas(g1 h0) + full-group o-chunks of g0 + transposes of g0
    #   hgk3: betas(g1 h1) + h0 o-chunks of g1 + finals of g0
    #   tail: h1 o-chunks of g1 (+add) + transposes/finals of g1
    out_sb[0] = outp.tile([P, NG * 4, C], FP16, tag="osb", name="outsb0")
    out_sb[1] = outp.tile([P, NG * 4, C], FP16, tag="osb", name="outsb1")
    AV = (V, A)
    for hgk in range(4):
        g, h = divmod(hgk, 2)
        for i in range(HGS):
            beta_exp(g, h * HGS + i, EXP_SCHED[hgk][i])
            if hgk < 2:
                late_proj(hgk, i)
            if hgk == 2:
                if i % 2 == 1:
                    o_chunk(0, i // 2, range(MT), "copy", copy_eng=AV[(i // 2) % 2])
                elif i >= 6 and i % 4 == 2:
                    pair_transpose(0, (i - 6) // 4)
            if hgk == 3:
                if i % 2 == 1:
                    o_chunk(
                        1, i // 2, range(HGS), "copy",
                        copy_eng=AV[(i // 2) % 2], opool=ps_pb,
                    )
                elif i == 0:
                    pair_transpose(0, 3)
                elif 2 <= i <= 8:
                    final_sub(0, i - 2)
                    final_sub(0, i - 1)
                elif i == 10:
                    half_out_dma(0, 0)
                    half_out_dma(0, 1)
    # tail: o-chunk pairs -> pool-add -> PE transpose, then finals with
    # quarter-granular output DMAs
    FP = [ps_pb, ps_m, ps_pb, ps_m, ps_pb, ps_m, ps_pb, ps_m]
    for p in range(4):
        o_chunk(
            1, 2 * p, range(HGS, MT), "second", copy_eng=AV[0],
            opool=ps_pb if p % 2 == 0 else ps_m,
        )
        o_chunk(
            1, 2 * p + 1, range(HGS, MT), "second", copy_eng=AV[1],
            opool=ps_pb if p % 2 == 0 else ps_m,
        )
        pair_transpose(1, p, via_pe=True, copy_eng=AV[p % 2])
    for s in range(8):
        final_sub(1, s, pool=FP[s])
        if s % 2 == 1:
            quarter_out_dma(1, s // 2)


def build_program():
    nc = bacc.Bacc(
        "TRN2",
        target_bir_lowering=False,
        debug=False,
        enable_asserts=False,
        num_devices=NCORES,
    )
    xh_d = nc.dram_tensor("xh", [NH, C], FP16, kind="ExternalInput").ap()
    xt_d = nc.dram_tensor("xt", [2, P, N], FP16, kind="ExternalInput").ap()
    xt0_d = nc.dram_tensor("xt0", [2, P, 512], FP16, kind="ExternalInput").ap()
    id_d = nc.dram_tensor("Ident", [P, P], BF16, kind="ExternalInput").ap()
    w3_d = nc.dram_tensor("W3", [2, P, 3 * D], FP16, kind="ExternalInput").ap()
    wv_d = nc.dram_tensor("WvDup", [P, C], BF16, kind="ExternalInput").ap()
    out_d = nc.dram_tensor("out", [NH, C], FP16, kind="ExternalOutput").ap()

    with tile.TileContext(nc) as tc:
        with ExitStack() as ctx:
            _body(ctx, tc, out_d, xh_d, xt_d, xt0_d, w3_d, wv_d, id_d)
    nc.compile()
    return nc


_CACHE = {}


def _get_program():
    if "nc" not in _CACHE:
        _CACHE["nc"] = build_program()
    return _CACHE["nc"]


def make_in_maps(inputs):
    x = np.ascontiguousarray(np.asarray(inputs["x"], np.float32)).reshape(B, N, C)
    gam = np.float32(np.asarray(inputs["gamma"], np.float32).reshape(()))
    w3 = np.empty((C, 3 * D), np.float16)  # [c, (Wf|Wg|Wh)]
    for j, nm in enumerate(("Wf", "Wg", "Wh")):
        w3[:, j * D : (j + 1) * D] = np.asarray(inputs[nm], np.float32).astype(
            np.float16
        )
    w3 = np.ascontiguousarray(w3.reshape(2, P, 3 * D))
    wv_dup = np.zeros((P, C), ml_dtypes.bfloat16)
    wv16 = (gam * np.asarray(inputs["Wv"], np.float32)).astype(ml_dtypes.bfloat16)
    wv_dup[0:D, :] = wv16
    wv_dup[64 : 64 + D, :] = wv16
    ident = np.eye(P, dtype=ml_dtypes.bfloat16)

    in_maps = []
    for c in range(NCORES):
        b, h = divmod(c, 2)
        if h == 0:
            xb = x[b]
        else:
            xb = np.concatenate([x[b, NH:], x[b, :NH]], axis=0)
        xt = np.ascontiguousarray(xb.T.astype(np.float16).reshape(2, P, N))
        in_maps.append(
            {
                "xh": np.ascontiguousarray(xb[:NH].astype(np.float16)),
                "xt": xt,
                "xt0": np.ascontiguousarray(xt[:, :, 0:512]),
                "Ident": ident,
                "W3": w3,
                "WvDup": wv_dup,
            }
        )
    return in_maps


def kernel(**inputs):
    global LAST_RESULTS
    nc = _get_program()
    in_maps = make_in_maps(inputs)
    res = run_bass_kernel_spmd(nc, in_maps, core_ids=list(range(NCORES)))
    LAST_RESULTS = res
    out = np.empty((B, N, C), np.float32)
    for c in range(NCORES):
        b, h = divmod(c, 2)
        out[b, h * NH : (h + 1) * NH] = res.results[c]["out"].astype(np.float32)
    return out.reshape(B, H, W, C)
